# revision 39
# baseline (speedup 1.0000x reference)
"""Trainium2 Bass kernel for nn_AttentionMechanism_41369124995539.

Math (per batch b):
    v[d]     = sum_e W[e, d] * hidden[b, e]          (bias `b` is a constant
    score[s] = sum_d cv[b, d, s] * v[d]  (+ const)    shift per batch -> drops
    attn     = softmax(mask(score))                   out of the softmax)
    ctx[d]   = sum_s cv[b, d, s] * attn[s]
Outputs broadcast over the seqlen axis on the host (the torch loop writes
identical values for every step).

Sharding: batch dim B=32 across 8 cores (4 batches/core); W replicated.
"""

import os
from contextlib import ExitStack

import numpy as np

import concourse.bass as bass
import concourse.masks as masks
import concourse.tile as tile
from concourse import bacc, mybir
from concourse.bass_utils import run_bass_kernel_spmd

B, D, S, SEQLEN = 32, 256, 8192, 64
NCORES = 8
BPC = B // NCORES          # batches per core
NCH = D // 128             # d-chunks of 128 partitions
NQ = 4                     # s-quarters
ST = S // NQ               # 2048 elements per quarter
NEG = -1.0e30

F32 = mybir.dt.float32
U8 = mybir.dt.uint8

SAFE_OPS = int(os.environ.get("KSAFE", "0"))


def build_nc(stage=5):
    nc = bacc.Bacc()
    cv = nc.declare_dram_parameter("cv", [BPC, D, S], F32, isOutput=False)
    hT = nc.declare_dram_parameter("hT", [D, BPC], F32, isOutput=False)
    W = nc.declare_dram_parameter("W", [D, D], F32, isOutput=False)
    mask = nc.declare_dram_parameter("mask", [BPC, S], U8, isOutput=False)
    attn_out = nc.declare_dram_parameter("attn", [BPC, S], F32, isOutput=True)
    ctxT_out = nc.declare_dram_parameter("ctxT", [128, BPC * NCH], F32, isOutput=True)

    with tile.TileContext(nc) as tc:
        with ExitStack() as ctx:
            _body(ctx, tc, cv, hT, W, mask, attn_out, ctxT_out, stage)
    nc.compile()
    return nc


def _body(ctx, tc, cv, hT, W, mask, attn_out, ctxT_out, stage=5):
    nc = tc.nc

    const = ctx.enter_context(tc.tile_pool(name="const", bufs=1))
    cvpool = ctx.enter_context(tc.tile_pool(name="cvp", bufs=12))
    ps = ctx.enter_context(tc.tile_pool(name="ps", bufs=4, space="PSUM"))
    pbc = ctx.enter_context(tc.tile_pool(name="pbc", bufs=2, space="PSUM"))
    sp = ctx.enter_context(tc.tile_pool(name="sp", bufs=2))
    small = ctx.enter_context(tc.tile_pool(name="small", bufs=16))
    accp = ctx.enter_context(tc.tile_pool(name="accp", bufs=4))

    mask_r = mask.rearrange("b (q f) -> b q f", q=NQ)
    attn_r = attn_out.rearrange("b (q f) -> b q f", q=NQ)

    # ---- setup: W, hiddenT, v columns -----------------------------------
    vz = []
    if stage >= 1:
        w_sb = const.tile([128, 2 * D], F32, tag="wsb")
        for e in range(NCH):
            nc.sync.dma_start(out=w_sb[:, e * D:(e + 1) * D],
                              in_=W[e * 128:(e + 1) * 128, :])
        ht_sb = const.tile([128, NCH * BPC], F32, tag="htsb")
        for e in range(NCH):
            nc.sync.dma_start(out=ht_sb[:, e * BPC:(e + 1) * BPC],
                              in_=hT[e * 128:(e + 1) * 128, :])

        # vT[d, b] = sum_e W[e, d] * hT[e, b], computed per d-chunk directly
        # in column-major layout, then scattered into a zero-padded weight
        # buffer: window w=(b*NCH+ch)*NQ+q is cols [4w, 4w+4) and holds
        # v[b, ch] at window-local column q, zeros elsewhere.
        vzbuf = const.tile([128, BPC * NCH * NQ * 4], F32, tag="vzbuf")
        nc.vector.memset(vzbuf, 0.0)
        for ch in range(NCH):
            vt_ps = ps.tile([128, BPC], F32, tag="ps", name=f"vtps{ch}")
            for e in range(NCH):
                nc.tensor.matmul(
                    vt_ps[:, :],
                    lhsT=w_sb[:, e * D + ch * 128: e * D + (ch + 1) * 128],
                    rhs=ht_sb[:, e * BPC:(e + 1) * BPC],
                    start=(e == 0), stop=(e == NCH - 1),
                )
            for b in range(BPC):
                for q in range(NQ):
                    w = (b * NCH + ch) * NQ + q
                    nc.scalar.copy(vzbuf[:, 4 * w + q:4 * w + q + 1],
                                   vt_ps[:, b:b + 1])
        vz = vzbuf

    if stage >= 4:
        ones4 = const.tile([BPC, 1], F32, tag="ones4")
        nc.vector.memset(ones4, 1.0)
        ident4 = const.tile([BPC, BPC], F32, tag="ident4")
        masks.make_identity(nc, ident4[:, :])
        ones_sb = const.tile([1, 128], F32, tag="ones")
        nc.vector.memset(ones_sb, 1.0)

    ctxcols = const.tile([128, BPC * NCH], F32, tag="ctxcols")
    if stage < 5:
        nc.vector.memset(ctxcols, 0.0)

    # ---- main pipeline over batches -------------------------------------
    pend = None  # deferred pass-2 closure for the previous batch

    for b in range(BPC):
        # stream this batch's contextvects into SBUF (stays resident for pass 2)
        cvt = {}
        for ch in range(NCH):
            for q in range(NQ):
                t = cvpool.tile([128, ST], F32, tag="cv")
                nc.sync.dma_start(out=t[:, :],
                                  in_=cv[b, ch * 128:(ch + 1) * 128, q * ST:(q + 1) * ST])
                cvt[ch, q] = t

        if stage < 1:
            # DMA-only smoke test: copy one cv tile row range to attn out
            t0 = cvt[0, 0]
            nc.sync.dma_start(out=attn_r[b], in_=t0[0:4, :])
            continue

        mu8 = sp.tile([BPC, ST], U8, tag="mu8")
        nc.sync.dma_start(out=mu8[:, :], in_=mask_r[b])
        mneg = sp.tile([BPC, ST], F32, tag="mneg", bufs=1)
        nc.scalar.mul(mneg[:, :], mu8[:, :], NEG)

        if stage < 2:
            # setup + mask conversion only: dump vz and mneg
            nc.sync.dma_start(out=attn_r[b][:, 0:128], in_=vz[0:4, :])
            nc.sync.dma_start(out=attn_r[b][:, 128:2048], in_=mneg[:, 128:2048])
            continue

        # scores: psum[q, j*512+f] accumulates v[b,ch] . cv[ch][q, j-tile]
        sc_ps = [ps.tile([BPC, 512], F32, tag="ps", name=f"scps{b}_{j}")
                 for j in range(4)]
        for j in range(4):
            for ch in range(NCH):
                for q in range(NQ):
                    w = (b * NCH + ch) * NQ + q
                    nc.tensor.matmul(
                        sc_ps[j][:, :],
                        lhsT=vz[:, 4 * w:4 * w + 4],
                        rhs=cvt[ch, q][:, j * 512:(j + 1) * 512],
                        start=(ch == 0 and q == 0),
                        stop=(ch == NCH - 1 and q == NQ - 1),
                    )
        sc_sb = sp.tile([BPC, ST], F32, tag="sc")
        for j in range(4):
            nc.scalar.copy(sc_sb[:, j * 512:(j + 1) * 512], sc_ps[j][:, :])

        if stage < 3:
            nc.sync.dma_start(out=attn_r[b], in_=sc_sb[:, :])
            continue

        # mask + per-row max (tensor_tensor_reduce faults on this runtime's
        # hardware, so plain add + reduce)
        rmax = small.tile([BPC, 1], F32, tag="sm")
        scm = sp.tile([BPC, ST], F32, tag="scm")
        nc.vector.tensor_add(scm[:, :], sc_sb[:, :], mneg[:, :])
        nc.vector.reduce_max(rmax[:, :], scm[:, :], axis=mybir.AxisListType.X)
        sc_sb = scm
        if stage < 4:
            negm = small.tile([BPC, 1], F32, tag="sm")
            nc.vector.tensor_scalar_mul(negm[:, :], rmax[:, :], -1.0)
            p_sb = sp.tile([BPC, ST], F32, tag="p")
            rsum = small.tile([BPC, 1], F32, tag="sm")
            nc.scalar.activation(p_sb[:, :], sc_sb[:, :],
                                 mybir.ActivationFunctionType.Exp,
                                 bias=negm[:, :], scale=1.0,
                                 accum_out=rsum[:, :])
            rl = small.tile([BPC, 1], F32, tag="sm")
            nc.vector.reciprocal(rl[:, :], rsum[:, :])
            nc.scalar.mul(p_sb[:, :], p_sb[:, :], rl[:, :])
            nc.sync.dma_start(out=attn_r[b], in_=p_sb[:, :])
            continue

        # global max of the 4 row maxes: transpose to one partition via PE,
        # reduce there, then broadcast -max back to 4 partitions via PE.
        rmax_t = ps.tile([1, BPC], F32, tag="ps", name=f"rmaxt{b}")
        nc.tensor.transpose(rmax_t[:, :], rmax[:, :], ident4[:, :])
        rmax_row = small.tile([1, BPC], F32, tag="smrow")
        nc.scalar.copy(rmax_row[:, :], rmax_t[:, :])
        negm1 = small.tile([1, 1], F32, tag="sm1")
        nc.vector.tensor_reduce(negm1[:, :], rmax_row[:, :],
                                axis=mybir.AxisListType.X,
                                op=mybir.AluOpType.max, negate=True)
        negm_ps = ps.tile([BPC, 1], F32, tag="ps", name=f"negmps{b}")
        nc.tensor.matmul(negm_ps[:, :], lhsT=ones_sb[:, 0:BPC],
                         rhs=negm1[:, :], start=True, stop=True)
        negm = small.tile([BPC, 1], F32, tag="sm")
        nc.scalar.copy(negm[:, :], negm_ps[:, :])

        # p = exp(score - max), fused row-sum
        p_sb = sp.tile([BPC, ST], F32, tag="p")
        rsum = small.tile([BPC, 1], F32, tag="sm")
        nc.scalar.activation(p_sb[:, :], sc_sb[:, :], mybir.ActivationFunctionType.Exp,
                             bias=negm[:, :], scale=1.0, accum_out=rsum[:, :])
        # total = sum over 4 partitions via K=4 ones-matmul; reciprocal;
        # broadcast back to 4 partitions.
        tot_ps = ps.tile([1, 1], F32, tag="ps", name=f"totps{b}")
        nc.tensor.matmul(tot_ps[:, :], lhsT=ones4[:, :], rhs=rsum[:, :],
                         start=True, stop=True)
        tot1 = small.tile([1, 1], F32, tag="sm1")
        nc.scalar.copy(tot1[:, :], tot_ps[:, :])
        rl1 = small.tile([1, 1], F32, tag="sm1")
        nc.vector.reciprocal(rl1[:, :], tot1[:, :])
        rl_ps = ps.tile([BPC, 1], F32, tag="ps", name=f"rlps{b}")
        nc.tensor.matmul(rl_ps[:, :], lhsT=ones_sb[:, 0:BPC], rhs=rl1[:, :],
                         start=True, stop=True)
        rl = small.tile([BPC, 1], F32, tag="sm")
        nc.scalar.copy(rl[:, :], rl_ps[:, :])

        nc.scalar.mul(p_sb[:, :], p_sb[:, :], rl[:, :])
        nc.sync.dma_start(out=attn_r[b], in_=p_sb[:, :])
        # flatten to one partition so pass-2 matmul rhs slices sit at base 0
        attn4 = sp.tile([1, S], F32, tag="attn4", bufs=1)
        nc.sync.dma_start(out=attn4[0:1, :], in_=p_sb[:, :])
        if stage < 5:
            continue

        # pass 2 (context accumulation) for the PREVIOUS batch is emitted
        # here so its PE work sits behind this batch's score matmuls.
        if pend is not None:
            pend()

        def make_pass2(b=b, cvt=cvt, attn4=attn4):
            def run():
                acc8 = {}
                for ch in range(NCH):
                    acc8[ch] = accp.tile([128, 8], F32, tag="acc8",
                                         name=f"acc8_{b}_{ch}")
                for q in range(NQ):
                    for h in range(2):
                        bc = pbc.tile([128, 1024], F32, tag="bc")
                        for j2 in range(2):
                            off = q * ST + h * 1024 + j2 * 512
                            nc.tensor.matmul(
                                bc[:, j2 * 512:(j2 + 1) * 512],
                                lhsT=ones_sb[:, :],
                                rhs=attn4[0:1, off: off + 512],
                                start=True, stop=True,
                            )
                        for ch in range(NCH):
                            # prod = cv * attn_bcast, clobbering the cv tile
                            # (its last reader); then ACT copy-with-accum
                            # reduces it along the free dim.
                            prod = cvt[ch, q][:, h * 1024:(h + 1) * 1024]
                            nc.vector.tensor_mul(prod, prod, bc[:, :])
                            k = q * 2 + h
                            nc.scalar.activation(
                                prod, prod, mybir.ActivationFunctionType.Copy,
                                accum_out=acc8[ch][:, k:k + 1])
                for ch in range(NCH):
                    nc.vector.reduce_sum(
                        ctxcols[:, b * NCH + ch: b * NCH + ch + 1],
                        acc8[ch][:, :], axis=mybir.AxisListType.X)
            return run

        pend = make_pass2()

    if pend is not None:
        pend()
    nc.sync.dma_start(out=ctxT_out[:, :], in_=ctxcols[:, :])


_NC_CACHE = None


def _get_nc():
    global _NC_CACHE
    if _NC_CACHE is None:
        _NC_CACHE = build_nc()
    return _NC_CACHE


def make_in_maps(hidden, contextvects, W, padding_mask):
    hidden = np.asarray(hidden, dtype=np.float32)
    contextvects = np.ascontiguousarray(np.asarray(contextvects, dtype=np.float32))
    W = np.ascontiguousarray(np.asarray(W, dtype=np.float32))
    mask_u8 = np.ascontiguousarray(
        np.asarray(padding_mask).reshape(B, S).astype(np.uint8))
    hT = np.ascontiguousarray(hidden.reshape(B, D).T)  # [D, B]

    in_maps = []
    for c in range(NCORES):
        sl = slice(c * BPC, (c + 1) * BPC)
        in_maps.append({
            "cv": np.ascontiguousarray(contextvects[sl]),
            "hT": np.ascontiguousarray(hT[:, sl]),
            "W": W,
            "mask": mask_u8[sl],
        })
    return in_maps


def assemble(results, seqlen):
    attn_full = np.concatenate([r["attn"] for r in results], axis=0)  # [B, S]
    ctx_parts = []
    for r in results:
        t = r["ctxT"]  # [128, BPC*NCH]; col b*NCH+ch = ctx[b, ch*128:(ch+1)*128]
        ctx_parts.append(np.ascontiguousarray(t.T).reshape(BPC, NCH * 128))
    ctx_full = np.concatenate(ctx_parts, axis=0)  # [B, D]
    n = int(seqlen)
    context = np.broadcast_to(ctx_full[None], (n, B, D))
    attentions = np.broadcast_to(attn_full[None], (n, B, S))
    return context, attentions


def run_spmd(in_maps, trace=False):
    nc = _get_nc()
    return run_bass_kernel_spmd(nc, in_maps, core_ids=list(range(NCORES)),
                                trace=trace)


def kernel(seqlen, hidden, contextvects, W, b, padding_mask):
    in_maps = make_in_maps(hidden, contextvects, W, padding_mask)
    res = run_spmd(in_maps, trace=False)
    return assemble(res.results, seqlen)


# revision 49
# speedup vs baseline: 464.0634x; 464.0634x over previous
"""Trainium2 Bass kernel for nn_AttentionMechanism_41369124995539.

Math (per batch b):
    v[d]     = sum_e W[e, d] * hidden[b, e]          (bias `b` is a constant
    score[s] = sum_d cv[b, d, s] * v[d]  (+ const)    shift per batch -> drops
    attn     = softmax(mask(score))                   out of the softmax)
    ctx[d]   = sum_s cv[b, d, s] * attn[s]
Outputs broadcast over the seqlen axis on the host (the torch loop writes
identical values for every step).

Sharding: batch dim B=32 across 8 cores (4 batches/core); W replicated.
"""

import os
from contextlib import ExitStack

import numpy as np

import concourse.bass as bass
import concourse.masks as masks
import concourse.tile as tile
from concourse import bacc, mybir
from concourse.bass_utils import run_bass_kernel_spmd

B, D, S, SEQLEN = 32, 256, 8192, 64
NCORES = 8
BPC = B // NCORES          # batches per core
NCH = D // 128             # d-chunks of 128 partitions
NQ = 4                     # s-quarters
ST = S // NQ               # 2048 elements per quarter
NEG = -1.0e30

F32 = mybir.dt.float32
BF16 = mybir.dt.bfloat16
U8 = mybir.dt.uint8

SAFE_OPS = int(os.environ.get("KSAFE", "0"))


def build_nc(stage=5):
    nc = bacc.Bacc()
    cv = nc.declare_dram_parameter("cv", [BPC, D, S], F32, isOutput=False)
    hT = nc.declare_dram_parameter("hT", [D, BPC], F32, isOutput=False)
    W = nc.declare_dram_parameter("W", [D, D], F32, isOutput=False)
    mask = nc.declare_dram_parameter("mask", [BPC, S], U8, isOutput=False)
    attn_out = nc.declare_dram_parameter("attn", [BPC, S], F32, isOutput=True)
    ctxT_out = nc.declare_dram_parameter("ctxT", [128, BPC * NCH], F32, isOutput=True)

    with tile.TileContext(nc) as tc:
        with ExitStack() as ctx:
            _body(ctx, tc, cv, hT, W, mask, attn_out, ctxT_out, stage)
    nc.compile()
    return nc


def _body(ctx, tc, cv, hT, W, mask, attn_out, ctxT_out, stage=5):
    nc = tc.nc

    const = ctx.enter_context(tc.tile_pool(name="const", bufs=1))
    cvpool = ctx.enter_context(tc.tile_pool(name="cvp", bufs=12))
    ps = ctx.enter_context(tc.tile_pool(name="ps", bufs=4, space="PSUM"))
    pbc = ctx.enter_context(tc.tile_pool(name="pbc", bufs=2, space="PSUM"))
    sp = ctx.enter_context(tc.tile_pool(name="sp", bufs=2))
    small = ctx.enter_context(tc.tile_pool(name="small", bufs=16))
    accp = ctx.enter_context(tc.tile_pool(name="accp", bufs=4))

    mask_r = mask.rearrange("b (q f) -> b q f", q=NQ)
    attn_r = attn_out.rearrange("b (q f) -> b q f", q=NQ)

    # ---- setup: W, hiddenT, v columns -----------------------------------
    vz = []
    if stage >= 1:
        w_sb = const.tile([128, 2 * D], F32, tag="wsb")
        for e in range(NCH):
            nc.sync.dma_start(out=w_sb[:, e * D:(e + 1) * D],
                              in_=W[e * 128:(e + 1) * 128, :])
        ht_sb = const.tile([128, NCH * BPC], F32, tag="htsb")
        for e in range(NCH):
            nc.sync.dma_start(out=ht_sb[:, e * BPC:(e + 1) * BPC],
                              in_=hT[e * 128:(e + 1) * 128, :])

        # vT[d, b] = sum_e W[e, d] * hT[e, b], computed per d-chunk directly
        # in column-major layout, then scattered into a zero-padded weight
        # buffer: window w=(b*NCH+ch)*NQ+q is cols [4w, 4w+4) and holds
        # v[b, ch] at window-local column q, zeros elsewhere.
        vzbuf = const.tile([128, BPC * NCH * NQ * 4], F32, tag="vzbuf")
        nc.vector.memset(vzbuf, 0.0)
        for ch in range(NCH):
            vt_ps = ps.tile([128, BPC], F32, tag="ps", name=f"vtps{ch}")
            for e in range(NCH):
                nc.tensor.matmul(
                    vt_ps[:, :],
                    lhsT=w_sb[:, e * D + ch * 128: e * D + (ch + 1) * 128],
                    rhs=ht_sb[:, e * BPC:(e + 1) * BPC],
                    start=(e == 0), stop=(e == NCH - 1),
                )
            for b in range(BPC):
                for q in range(NQ):
                    w = (b * NCH + ch) * NQ + q
                    nc.scalar.copy(vzbuf[:, 4 * w + q:4 * w + q + 1],
                                   vt_ps[:, b:b + 1])
        vz = vzbuf

    if stage >= 4:
        ones4 = const.tile([BPC, 1], F32, tag="ones4")
        nc.vector.memset(ones4, 1.0)
        ident4 = const.tile([BPC, BPC], F32, tag="ident4")
        masks.make_identity(nc, ident4[:, :])
        ones_sb = const.tile([1, 128], F32, tag="ones")
        nc.vector.memset(ones_sb, 1.0)
        # selector weights: eq[q] is [BPC, 128] bf16 with row q all-ones —
        # K=4 matmul with rhs=[4, N] attn rows replicates row q across all
        # 128 output partitions without any partition-collapse DMA.
        eqsel = const.tile([BPC, NQ * 128], BF16, tag="eqsel")
        nc.gpsimd.memset(eqsel, 0.0)
        # iota[p, (q, j)] = p - q; where != 0 keep 0, where == 0 fill 1.0
        nc.gpsimd.affine_select(
            out=eqsel[:, :], in_=eqsel[:, :],
            compare_op=mybir.AluOpType.not_equal, fill=1.0,
            base=0, pattern=[[-1, NQ], [0, 128]], channel_multiplier=1)

    ctxcols = const.tile([128, BPC * NCH], F32, tag="ctxcols")
    if stage < 5:
        nc.vector.memset(ctxcols, 0.0)

    # ---- main pipeline over batches -------------------------------------
    pend = None  # deferred pass-2 closure for the previous batch

    for b in range(BPC):
        # stream this batch's contextvects into SBUF (stays resident for pass 2)
        cvt = {}
        for ch in range(NCH):
            for q in range(NQ):
                t = cvpool.tile([128, ST], F32, tag="cv")
                nc.sync.dma_start(out=t[:, :],
                                  in_=cv[b, ch * 128:(ch + 1) * 128, q * ST:(q + 1) * ST])
                cvt[ch, q] = t

        if stage < 1:
            # DMA-only smoke test: copy one cv tile row range to attn out
            t0 = cvt[0, 0]
            nc.sync.dma_start(out=attn_r[b], in_=t0[0:4, :])
            continue

        mu8 = sp.tile([BPC, ST], U8, tag="mu8", bufs=1)
        nc.sync.dma_start(out=mu8[:, :], in_=mask_r[b])
        mneg = sp.tile([BPC, ST], F32, tag="mneg", bufs=1)
        nc.scalar.mul(mneg[:, :], mu8[:, :], NEG)

        if stage < 2:
            # setup + mask conversion only: dump vz and mneg
            nc.sync.dma_start(out=attn_r[b][:, 0:128], in_=vz[0:4, :])
            nc.sync.dma_start(out=attn_r[b][:, 128:2048], in_=mneg[:, 128:2048])
            continue

        # scores: psum[q, j*512+f] accumulates v[b,ch] . cv[ch][q, j-tile]
        sc_ps = [ps.tile([BPC, 512], F32, tag="ps", name=f"scps{b}_{j}")
                 for j in range(4)]
        for j in range(4):
            for ch in range(NCH):
                for q in range(NQ):
                    w = (b * NCH + ch) * NQ + q
                    nc.tensor.matmul(
                        sc_ps[j][:, :],
                        lhsT=vz[:, 4 * w:4 * w + 4],
                        rhs=cvt[ch, q][:, j * 512:(j + 1) * 512],
                        start=(ch == 0 and q == 0),
                        stop=(ch == NCH - 1 and q == NQ - 1),
                    )
        sc_sb = sp.tile([BPC, ST], F32, tag="sc")
        for j in range(4):
            nc.scalar.copy(sc_sb[:, j * 512:(j + 1) * 512], sc_ps[j][:, :])

        if stage < 3:
            nc.sync.dma_start(out=attn_r[b], in_=sc_sb[:, :])
            continue

        # mask + per-row max (tensor_tensor_reduce faults on this runtime's
        # hardware, so plain add + reduce)
        rmax = small.tile([BPC, 1], F32, tag="sm")
        scm = sp.tile([BPC, ST], F32, tag="scm")
        nc.vector.tensor_add(scm[:, :], sc_sb[:, :], mneg[:, :])
        nc.vector.reduce_max(rmax[:, :], scm[:, :], axis=mybir.AxisListType.X)
        sc_sb = scm
        if stage < 4:
            negm = small.tile([BPC, 1], F32, tag="sm")
            nc.vector.tensor_scalar_mul(negm[:, :], rmax[:, :], -1.0)
            p_sb = sp.tile([BPC, ST], F32, tag="p")
            rsum = small.tile([BPC, 1], F32, tag="sm")
            nc.scalar.activation(p_sb[:, :], sc_sb[:, :],
                                 mybir.ActivationFunctionType.Exp,
                                 bias=negm[:, :], scale=1.0,
                                 accum_out=rsum[:, :])
            rl = small.tile([BPC, 1], F32, tag="sm")
            nc.vector.reciprocal(rl[:, :], rsum[:, :])
            nc.scalar.mul(p_sb[:, :], p_sb[:, :], rl[:, :])
            nc.sync.dma_start(out=attn_r[b], in_=p_sb[:, :])
            continue

        # global max of the 4 row maxes: transpose to one partition via PE,
        # reduce there, then broadcast -max back to 4 partitions via PE.
        rmax_t = ps.tile([1, BPC], F32, tag="ps", name=f"rmaxt{b}")
        nc.tensor.transpose(rmax_t[:, :], rmax[:, :], ident4[:, :])
        rmax_row = small.tile([1, BPC], F32, tag="smrow")
        nc.scalar.copy(rmax_row[:, :], rmax_t[:, :])
        negm1 = small.tile([1, 1], F32, tag="sm1")
        nc.vector.tensor_reduce(negm1[:, :], rmax_row[:, :],
                                axis=mybir.AxisListType.X,
                                op=mybir.AluOpType.max, negate=True)
        negm_ps = ps.tile([BPC, 1], F32, tag="ps", name=f"negmps{b}")
        nc.tensor.matmul(negm_ps[:, :], lhsT=ones_sb[:, 0:BPC],
                         rhs=negm1[:, :], start=True, stop=True)
        negm = small.tile([BPC, 1], F32, tag="sm")
        nc.scalar.copy(negm[:, :], negm_ps[:, :])

        # p = exp(score - max), fused row-sum
        p_sb = sp.tile([BPC, ST], F32, tag="p")
        rsum = small.tile([BPC, 1], F32, tag="sm")
        nc.scalar.activation(p_sb[:, :], sc_sb[:, :], mybir.ActivationFunctionType.Exp,
                             bias=negm[:, :], scale=1.0, accum_out=rsum[:, :])
        # total = sum over 4 partitions via K=4 ones-matmul; reciprocal;
        # broadcast back to 4 partitions.
        tot_ps = ps.tile([1, 1], F32, tag="ps", name=f"totps{b}")
        nc.tensor.matmul(tot_ps[:, :], lhsT=ones4[:, :], rhs=rsum[:, :],
                         start=True, stop=True)
        tot1 = small.tile([1, 1], F32, tag="sm1")
        nc.scalar.copy(tot1[:, :], tot_ps[:, :])
        rl1 = small.tile([1, 1], F32, tag="sm1")
        nc.vector.reciprocal(rl1[:, :], tot1[:, :])
        rl_ps = ps.tile([BPC, 1], F32, tag="ps", name=f"rlps{b}")
        nc.tensor.matmul(rl_ps[:, :], lhsT=ones_sb[:, 0:BPC], rhs=rl1[:, :],
                         start=True, stop=True)
        rl = small.tile([BPC, 1], F32, tag="sm")
        nc.scalar.copy(rl[:, :], rl_ps[:, :])

        nc.scalar.mul(p_sb[:, :], p_sb[:, :], rl[:, :])
        nc.sync.dma_start(out=attn_r[b], in_=p_sb[:, :])
        # split attn into bf16 hi+lo (exact to ~2^-17) so the pass-2
        # broadcast matmuls can run at bf16 rate while staying fp32-accurate
        # in the PSUM accumulation; flatten each to one partition so matmul
        # rhs slices sit at base partition 0.
        hi_sb = sp.tile([BPC, ST], BF16, tag="hi", bufs=2)
        nc.scalar.copy(hi_sb[:, :], p_sb[:, :])
        lo_sb = sp.tile([BPC, ST], BF16, tag="lo", bufs=2)
        nc.vector.tensor_tensor(out=lo_sb[:, :], in0=p_sb[:, :],
                                in1=hi_sb[:, :],
                                op=mybir.AluOpType.subtract)
        if stage < 5:
            continue

        # pass 2 (context accumulation) for the PREVIOUS batch is emitted
        # here so its PE work sits behind this batch's score matmuls.
        if pend is not None:
            pend()

        def make_pass2(b=b, cvt=cvt, hi_sb=hi_sb, lo_sb=lo_sb):
            def run():
                acc8 = {}
                for ch in range(NCH):
                    acc8[ch] = accp.tile([128, 8], F32, tag="acc8",
                                         name=f"acc8_{b}_{ch}")
                for q in range(NQ):
                    sel = eqsel[:, q * 128:(q + 1) * 128]
                    for h in range(2):
                        bc = pbc.tile([128, 1024], F32, tag="bc")
                        for j2 in range(2):
                            off = h * 1024 + j2 * 512
                            nc.tensor.matmul(
                                bc[:, j2 * 512:(j2 + 1) * 512],
                                lhsT=sel,
                                rhs=hi_sb[:, off: off + 512],
                                start=True, stop=False,
                            )
                            nc.tensor.matmul(
                                bc[:, j2 * 512:(j2 + 1) * 512],
                                lhsT=sel,
                                rhs=lo_sb[:, off: off + 512],
                                start=False, stop=True,
                            )
                        for ch in range(NCH):
                            # prod = cv * attn_bcast, clobbering the cv tile
                            # (its last reader); then ACT copy-with-accum
                            # reduces it along the free dim.
                            prod = cvt[ch, q][:, h * 1024:(h + 1) * 1024]
                            nc.vector.tensor_mul(prod, prod, bc[:, :])
                            k = q * 2 + h
                            nc.scalar.activation(
                                prod, prod, mybir.ActivationFunctionType.Copy,
                                accum_out=acc8[ch][:, k:k + 1])
                for ch in range(NCH):
                    nc.vector.reduce_sum(
                        ctxcols[:, b * NCH + ch: b * NCH + ch + 1],
                        acc8[ch][:, :], axis=mybir.AxisListType.X)
            return run

        pend = make_pass2()

    if pend is not None:
        pend()
    nc.sync.dma_start(out=ctxT_out[:, :], in_=ctxcols[:, :])


_NC_CACHE = None


def _get_nc():
    global _NC_CACHE
    if _NC_CACHE is None:
        _NC_CACHE = build_nc()
    return _NC_CACHE


def make_in_maps(hidden, contextvects, W, padding_mask):
    hidden = np.asarray(hidden, dtype=np.float32)
    contextvects = np.ascontiguousarray(np.asarray(contextvects, dtype=np.float32))
    W = np.ascontiguousarray(np.asarray(W, dtype=np.float32))
    mask_u8 = np.ascontiguousarray(
        np.asarray(padding_mask).reshape(B, S).astype(np.uint8))
    hT = np.ascontiguousarray(hidden.reshape(B, D).T)  # [D, B]

    in_maps = []
    for c in range(NCORES):
        sl = slice(c * BPC, (c + 1) * BPC)
        in_maps.append({
            "cv": np.ascontiguousarray(contextvects[sl]),
            "hT": np.ascontiguousarray(hT[:, sl]),
            "W": W,
            "mask": mask_u8[sl],
        })
    return in_maps


def assemble(results, seqlen):
    attn_full = np.concatenate([r["attn"] for r in results], axis=0)  # [B, S]
    ctx_parts = []
    for r in results:
        t = r["ctxT"]  # [128, BPC*NCH]; col b*NCH+ch = ctx[b, ch*128:(ch+1)*128]
        ctx_parts.append(np.ascontiguousarray(t.T).reshape(BPC, NCH * 128))
    ctx_full = np.concatenate(ctx_parts, axis=0)  # [B, D]
    n = int(seqlen)
    context = np.broadcast_to(ctx_full[None], (n, B, D))
    attentions = np.broadcast_to(attn_full[None], (n, B, S))
    return context, attentions


def run_spmd(in_maps, trace=False):
    nc = _get_nc()
    return run_bass_kernel_spmd(nc, in_maps, core_ids=list(range(NCORES)),
                                trace=trace)


def kernel(seqlen, hidden, contextvects, W, b, padding_mask):
    in_maps = make_in_maps(hidden, contextvects, W, padding_mask)
    res = run_spmd(in_maps, trace=False)
    return assemble(res.results, seqlen)
